# revision 46
# baseline (speedup 1.0000x reference)
"""CenterLoss kernel for Trainium2 (Bass/Tile), 8-core data-parallel.

loss = sum_i ||x_i - centers[labels_i]||^2
  x: (65536, 512) f32, labels: (65536,) int, centers: (512, 512) f32

Per-core plan (8192 rows each), using the expansion
  loss = sum x^2 - 2*sum_{c,d} S[c,d]*centers[c,d] + sum_c count_c*||C_c||^2
with S = onehot(labels)^T @ x on the PE (DoubleRow fp8 matmuls). The third
term needs only labels+centers, so it is computed on the host. The device
computes
  r1 = sum x^2   -- early chunks: Gram diag-blocks on the PE (G_m = Xm^T Xm
                    accumulated in PSUM, diagonal extracted with an identity
                    STT); later chunks: ACT Square-accum from the f32 tiles
  r2 = -2*sum S.*C  -- per-class-chunk DVE STT over PSUM against SBUF C
and reduces r1+r2 across partitions with a ones-vector matmul so the output
is a single [1,1] scalar (one DMA descriptor -> short completion tail).

DMA strategy (all HWDGE; SWDGE casting DMA starves the HWDGE rings and its
Q7 descriptor generation is ~4x too slow): x is striped across BOTH HWDGE
rings. One ring alone tops out near ~320 GB/s, together they reach the
~425 GB/s fabric limit. The sync ring carries many small/medium chunks
(trigger-semaphore-pool blocking is harmless on the otherwise idle sync
engine); the scalar ring carries exactly 3 big chunks triggered upfront so
the ACT engine never blocks on a trigger. centers ride the idle gpsimd
SWDGE queue (needed only at the tail). A merged aux tensor (iota+labels+
consts) leads the sync ring so one-hot building can start by ~8us.

f32->fp8 casts are split DVE tensor_copy (2x) / ACT activation-Copy to
balance the engines; warmup matmuls lift the PE HAM throttle before the
first real group.
"""

import sys

import numpy as np

sys.path.insert(0, "/opt/trn_rl_repo")

N_CORES = 8
B = 65536
D = 512
B_L = B // N_CORES  # 8192 rows per core
NCH = D // 128  # 4 class chunks

# x chunks in compute (arrival) order: (rows, ring). Ring cumulative bytes
# are paced so each ring delivers just ahead of the PE's consumption.
CHUNKS = [
    (256, "sync"),
    (512, "sync"),
    (1024, "sync"),
    (1024, "sync"),
    (1024, "sync"),
    (1024, "sync"),
    (1024, "sync"),
    (1024, "sync"),
    (512, "sync"),
    (512, "sync"),
    (256, "sync"),
]
CHUNK_ROWS = [r for r, _ in CHUNKS]
assert sum(CHUNK_ROWS) == B_L
assert all((r // 128) % 2 == 0 for r in CHUNK_ROWS)
N_CHUNKS = len(CHUNKS)

# chunks whose sum(x^2) comes from PE Gram diag-blocks (early chunks, while
# the PE still has slack); the rest use ACT Square-accum on the f32 data
GRAM_CHUNKS = {2, 3, 4, 5}
# chunks whose f32->fp8 cast runs on DVE (rest on ACT)
DVE_CAST_CHUNKS = {7, 8}

N_WARMUP_MM = 8  # junk matmuls to lift the PE HAM throttle before real work

AUX_COLS = 512  # fp16 iota row; labels ride a small f32 tensor

_CACHE = {}


def _build():
    """Trace the Bass/Tile program once; returns the compiled Bacc module."""
    if "nc" in _CACHE:
        return _CACHE["nc"]

    import concourse.bacc as bacc
    import concourse.mybir as mybir
    import concourse.tile as tile

    f32 = mybir.dt.float32
    fp8 = mybir.dt.float8e4

    nc = bacc.Bacc("TRN2", debug=False, num_devices=N_CORES)
    x_t = nc.dram_tensor("x", [B_L, D], f32, kind="ExternalInput")
    aux_t = nc.dram_tensor("aux", [128, AUX_COLS], mybir.dt.float16, kind="ExternalInput")
    lab_t = nc.dram_tensor("labx", [128, B_L // 128 + 1], f32, kind="ExternalInput")
    c_t = nc.dram_tensor("centers", [D, D], f32, kind="ExternalInput")
    out_t = nc.dram_tensor("out", [1, 1], f32, kind="ExternalOutput")

    qcs = [r // 128 for r in CHUNK_ROWS]
    toff = [sum(qcs[:i]) for i in range(N_CHUNKS)]  # labf col offset
    goff = [sum(q // 2 for q in qcs[:i]) for i in range(N_CHUNKS)]
    n_groups = B_L // 256  # 32 DoubleRow matmul groups
    gram_groups = sorted(
        goff[ci] + j for ci in GRAM_CHUNKS for j in range(qcs[ci] // 2)
    )
    sumsq_chunks = [ci for ci in range(N_CHUNKS) if ci not in GRAM_CHUNKS]

    with tile.TileContext(nc) as tc:
        with (
            tc.tile_pool(name="misc", bufs=1) as misc_pool,
            tc.tile_pool(name="psum", bufs=1, space="PSUM") as psum_pool,
        ):
            iota_sb16 = misc_pool.tile([128, D], mybir.dt.float16)
            lab_sb = misc_pool.tile([128, B_L // 128 + 1], f32)
            cent_sb = misc_pool.tile([128, NCH, D], f32)
            x32 = [
                None
                if CHUNKS[i][1] == "gpsimd"
                else misc_pool.tile([128, q, D], f32, name=f"x32_{i}")
                for i, q in enumerate(qcs)
            ]
            x8 = [
                misc_pool.tile([128, q, D], fp8, name=f"x8_{i}")
                for i, q in enumerate(qcs)
            ]

            labf_sb = lab_sb[:, 0 : B_L // 128]
            pidx_sb = lab_sb[:, B_L // 128 : B_L // 128 + 1]
            ones_sb = misc_pool.tile([128, 1], f32)

            acc_x2 = misc_pool.tile([128, len(sumsq_chunks)], f32)
            r2acc = misc_pool.tile([128, NCH], f32)
            junk_dve = misc_pool.tile([128, 1], f32)
            junk_act = misc_pool.tile([128, 1], f32)
            r1 = misc_pool.tile([128, 1], f32)
            r1g = misc_pool.tile([128, 1], f32)
            r2 = misc_pool.tile([128, 1], f32)
            total = misc_pool.tile([128, 1], f32)
            res_sb = misc_pool.tile([128, 1], f32)
            eye8 = misc_pool.tile([128, 1, 128], f32)
            warm8 = misc_pool.tile([128, 2, D], fp8)

            S_all = psum_pool.tile([128, NCH, D], f32, name="S_all")
            S_ps = [S_all[:, c, :] for c in range(NCH)]
            # one bank per Gram block: start=True clears has_written at BANK
            # granularity, so accumulation regions must not share a bank
            G_wide = psum_pool.tile([128, NCH, D], f32, name="G_wide")
            G_blk = [G_wide[:, m, 0:128] for m in range(NCH)]
            warm_ps = G_wide[:, 0, :]  # warmup scribbles are cleared by the
            red_ps = G_wide[0:1, 0, 0:2]  # real G start; reduce runs last

            # --- DMA triggers, emission order = semaphore allocation order.
            # sync ring: aux + first sync chunks; scalar ring: its 3 chunks
            # (must never block the ACT engine); then centers on the idle
            # gpsimd queue and the remaining sync chunks (a blocked trigger
            # on the sync/gpsimd engines is harmless).
            x_ap = x_t.ap()
            chunk_lo = []
            lo = 0
            for rows, _ in CHUNKS:
                chunk_lo.append(lo)
                lo += rows

            def x_src(ci):
                lo, rows = chunk_lo[ci], CHUNK_ROWS[ci]
                return x_ap[lo : lo + rows, :].rearrange("(p q) d -> p q d", p=128)

            def x_src_cast(ci):
                # 4-row (8KB) descriptor granularity for the SWDGE cast DMA:
                # big enough that Q7 descriptor-gen keeps up, small enough
                # that packet-granular round-robin shares fairly with HWDGE
                lo, rows = chunk_lo[ci], CHUNK_ROWS[ci]
                return x_ap[lo : lo + rows, :].rearrange(
                    "(p q f) d -> p q (f d)", p=128, f=4
                )

            # emission order = semaphore allocation order: the first ~7
            # in-flight DMAs get sems without blocking; later triggers
            # block only their own (idle) issuing engine until a sem frees
            sync_x = [ci for ci, (_, ring) in enumerate(CHUNKS) if ring == "sync"]
            gps_x = [ci for ci, (_, ring) in enumerate(CHUNKS) if ring == "gpsimd"]
            nc.sync.dma_start(iota_sb16[:], aux_t.ap())
            nc.sync.dma_start(lab_sb[:], lab_t.ap())
            for ci in sync_x[:4]:
                nc.sync.dma_start(x32[ci][:], x_src(ci))
            for ci in gps_x:
                nc.gpsimd.dma_start(
                    x8[ci][:].rearrange("p q d -> p (q d)").rearrange(
                        "p (q v) -> p q v", v=4 * D
                    ),
                    x_src_cast(ci),
                )
            for ci in sync_x[4:]:
                nc.sync.dma_start(x32[ci][:], x_src(ci))
            nc.gpsimd.dma_start(
                cent_sb[:], c_t.ap().rearrange("(n p) d -> p n d", p=128)
            )

            # --- PE warmup on a memset tile (HAM un-throttle) + identity
            nc.vector.memset(warm8[:], 0.0)
            nc.vector.memset(ones_sb[:], 1.0)
            for _ in range(N_WARMUP_MM):
                nc.tensor.matmul(
                    warm_ps[:],
                    lhsT=warm8[:, :, 0:128],
                    rhs=warm8[:],
                    start=True,
                    stop=True,
                    perf_mode=mybir.MatmulPerfMode.DoubleRow,
                )
            # eye8[p, m, q] = (q == p), for extracting Gram diag blocks
            if GRAM_CHUNKS:
                nc.vector.tensor_scalar(
                    out=eye8[:, 0, :],
                    in0=iota_sb16[:, 0:128],
                    scalar1=pidx_sb,
                    scalar2=None,
                    op0=mybir.AluOpType.is_equal,
                )

            # --- main pipeline, chunk order = arrival order
            for ci, (rows, ring) in enumerate(CHUNKS):
                qc = qcs[ci]
                # f32 -> fp8 cast per 512-row slab on the assigned engine
                if ring != "gpsimd":
                    n_sl = (qc + 3) // 4
                    for k in range(n_sl):
                        sl = slice(4 * k, min(4 * k + 4, qc))
                        if ci in DVE_CAST_CHUNKS:
                            nc.vector.tensor_copy(
                                x8[ci][:, sl, :], x32[ci][:, sl, :]
                            )
                        else:
                            nc.scalar.activation(
                                x8[ci][:, sl, :],
                                x32[ci][:, sl, :],
                                mybir.ActivationFunctionType.Copy,
                            )
                if ci not in GRAM_CHUNKS:
                    # sum(x^2) for the chunk on ACT (exact f32)
                    x_sq = x8[ci] if ring == "gpsimd" else x32[ci]
                    x_flat = x_sq[:].rearrange("p q d -> p (q d)")
                    col = sumsq_chunks.index(ci)
                    nc.scalar.activation(
                        junk_act[:].broadcast_to(x_flat.shape),
                        x_flat,
                        mybir.ActivationFunctionType.Square,
                        accum_out=acc_x2[:, col : col + 1],
                    )
                # per 256-row group: one-hot build (DVE) + 4 DoubleRow
                # matmuls into S (+ Gram diag-blocks on the Gram chunks)
                for j in range(qc // 2):
                    g = goff[ci] + j
                    oh = misc_pool.tile([128, 2, D], fp8, tag="oh", bufs=24)
                    for u in range(2):
                        tcol = 2 * g + u
                        nc.vector.tensor_scalar(
                            out=oh[:, u, :],
                            in0=iota_sb16[:],
                            scalar1=labf_sb[:, tcol : tcol + 1],
                            scalar2=None,
                            op0=mybir.AluOpType.is_equal,
                        )
                    for c in range(NCH):
                        nc.tensor.matmul(
                            S_ps[c],
                            lhsT=oh[:, :, c * 128 : (c + 1) * 128],
                            rhs=x8[ci][:, 2 * j : 2 * j + 2, :],
                            start=g == 0,
                            stop=g == n_groups - 1,
                            perf_mode=mybir.MatmulPerfMode.DoubleRow,
                        )
                    if ci in GRAM_CHUNKS:
                        for m in range(NCH):
                            xs = x8[ci][:, 2 * j : 2 * j + 2, m * 128 : (m + 1) * 128]
                            nc.tensor.matmul(
                                G_blk[m],
                                lhsT=xs,
                                rhs=xs,
                                start=g == gram_groups[0],
                                stop=g == gram_groups[-1],
                                perf_mode=mybir.MatmulPerfMode.DoubleRow,
                            )

            # --- tail
            # r1g = sum of Gram diagonals (identity-masked STT over PSUM)
            if GRAM_CHUNKS:
                G_flat = G_wide[:, :, 0:128]
                nc.vector.scalar_tensor_tensor(
                    out=junk_dve[:].broadcast_to(G_flat.shape),
                    in0=G_flat,
                    scalar=1.0,
                    in1=eye8[:].broadcast_to([128, NCH, 128]),
                    op0=mybir.AluOpType.bypass,
                    op1=mybir.AluOpType.mult,
                    accum_out=r1g[:],
                )
            # r2_c = -2*sum_d S[c,d]*C[c,d], one STT per class chunk
            for c in range(NCH):
                nc.vector.scalar_tensor_tensor(
                    out=junk_dve[:].broadcast_to(S_ps[c].shape),
                    in0=S_ps[c],
                    scalar=-2.0,
                    in1=cent_sb[:, c, :],
                    op0=mybir.AluOpType.mult,
                    op1=mybir.AluOpType.mult,
                    accum_out=r2acc[:, c : c + 1],
                )
            nc.vector.tensor_reduce(
                r1[:], acc_x2[:], axis=mybir.AxisListType.X, op=mybir.AluOpType.add
            )
            nc.vector.tensor_reduce(
                r2[:], r2acc[:], axis=mybir.AxisListType.X, op=mybir.AluOpType.add
            )
            nc.vector.tensor_tensor(total[:], r1[:], r2[:], op=mybir.AluOpType.add)
            if GRAM_CHUNKS:
                nc.vector.tensor_tensor(
                    total[:], total[:], r1g[:], op=mybir.AluOpType.add
                )
            # cross-partition reduce on the PE: [1,1] = total^T @ ones
            nc.tensor.matmul(
                red_ps[0:1, 0:1],
                lhsT=total[:],
                rhs=ones_sb,
                start=True,
                stop=True,
                skip_group_check=True,
            )
            nc.vector.tensor_copy(res_sb[0:1, 0:1], red_ps[0:1, 0:1])
            nc.sync.dma_start(out_t.ap(), res_sb[0:1, 0:1])

    nc.compile()
    _CACHE["nc"] = nc
    return nc


def _prep_inputs(x, labels, centers):
    """Shard full inputs into the 8 per-core input maps."""
    x = np.asarray(x, dtype=np.float32)
    labels = np.asarray(labels)
    centers = np.ascontiguousarray(np.asarray(centers, dtype=np.float32))
    in_maps = []
    for cidx in range(N_CORES):
        xs = np.ascontiguousarray(x[cidx * B_L : (cidx + 1) * B_L])
        lab = np.asarray(labels[cidx * B_L : (cidx + 1) * B_L], dtype=np.int64)
        # labf[p, t]: label of the row that lands at (partition p, q-col t),
        # chunk ci contributing qc = rows/128 q-cols, row = lo + p*qc + qq
        cols = []
        lo = 0
        for rows in CHUNK_ROWS:
            qc = rows // 128
            cols.append(lab[lo : lo + rows].reshape(128, qc))
            lo += rows
        labf = np.concatenate(cols, axis=1).astype(np.float32)
        labx = np.zeros((128, B_L // 128 + 1), dtype=np.float32)
        labx[:, 0 : B_L // 128] = labf
        labx[:, B_L // 128] = np.arange(128, dtype=np.float32)  # pidx
        in_maps.append(
            {
                "x": xs,
                "aux": np.tile(np.arange(D, dtype=np.float16), (128, 1)),
                "labx": np.ascontiguousarray(labx),
                "centers": centers,
            }
        )
    return in_maps


def _run(x, labels, centers, trace=False):
    from concourse import bass_utils

    nc = _build()
    in_maps = _prep_inputs(x, labels, centers)
    res = bass_utils.run_bass_kernel_spmd(
        nc, in_maps, core_ids=list(range(N_CORES)), trace=trace
    )
    total = np.float64(0.0)
    for r in res.results:
        total += np.sum(r["out"].astype(np.float64))
    # r3 = sum_c count_c * ||C_c||^2 from the labels histogram (host-side;
    # needs only labels+centers, no x)
    lab = np.asarray(labels).astype(np.int64)
    bc = np.bincount(lab, minlength=D).astype(np.float64)
    c64 = np.asarray(centers, dtype=np.float64)
    total += float(np.dot(bc, np.einsum("cd,cd->c", c64, c64)))
    return np.array(total, dtype=np.float32), res


def kernel(x, labels, centers):
    out, _ = _run(x, labels, centers, trace=False)
    return out


def kernel_traced(x, labels, centers):
    return _run(x, labels, centers, trace=True)


# revision 47
# speedup vs baseline: 1.1961x; 1.1961x over previous
"""CenterLoss kernel for Trainium2 (Bass/Tile), 8-core data-parallel.

loss = sum_i ||x_i - centers[labels_i]||^2
  x: (65536, 512) f32, labels: (65536,) int, centers: (512, 512) f32

Per-core plan (8192 rows each), using the expansion
  loss = sum x^2 - 2*sum_{c,d} S[c,d]*centers[c,d] + sum_c count_c*||C_c||^2
with S = onehot(labels)^T @ x on the PE (DoubleRow fp8 matmuls). The third
term needs only labels+centers, so it is computed on the host. The device
computes
  r1 = sum x^2   -- early chunks: Gram diag-blocks on the PE (G_m = Xm^T Xm
                    accumulated in PSUM, diagonal extracted with an identity
                    STT); later chunks: ACT Square-accum from the f32 tiles
  r2 = -2*sum S.*C  -- per-class-chunk DVE STT over PSUM against SBUF C
and reduces r1+r2 across partitions with a ones-vector matmul so the output
is a single [1,1] scalar (one DMA descriptor -> short completion tail).

DMA strategy (all HWDGE; SWDGE casting DMA starves the HWDGE rings and its
Q7 descriptor generation is ~4x too slow): x is striped across BOTH HWDGE
rings. One ring alone tops out near ~320 GB/s, together they reach the
~425 GB/s fabric limit. The sync ring carries many small/medium chunks
(trigger-semaphore-pool blocking is harmless on the otherwise idle sync
engine); the scalar ring carries exactly 3 big chunks triggered upfront so
the ACT engine never blocks on a trigger. centers ride the idle gpsimd
SWDGE queue (needed only at the tail). A merged aux tensor (iota+labels+
consts) leads the sync ring so one-hot building can start by ~8us.

f32->fp8 casts are split DVE tensor_copy (2x) / ACT activation-Copy to
balance the engines; warmup matmuls lift the PE HAM throttle before the
first real group.
"""

import sys

import numpy as np

sys.path.insert(0, "/opt/trn_rl_repo")

N_CORES = 8
B = 65536
D = 512
B_L = B // N_CORES  # 8192 rows per core
NCH = D // 128  # 4 class chunks

# x chunks in compute (arrival) order: (rows, ring). Ring cumulative bytes
# are paced so each ring delivers just ahead of the PE's consumption.
CHUNKS = [
    (256, "sync"),
    (512, "sync"),
    (1024, "sync"),
    (1024, "sync"),
    (1024, "sync"),
    (1024, "sync"),
    (1024, "sync"),
    (1024, "sync"),
    (512, "sync"),
    (512, "sync"),
    (256, "sync"),
]
CHUNK_ROWS = [r for r, _ in CHUNKS]
assert sum(CHUNK_ROWS) == B_L
assert all((r // 128) % 2 == 0 for r in CHUNK_ROWS)
N_CHUNKS = len(CHUNKS)

# chunks whose sum(x^2) comes from PE Gram diag-blocks (early chunks, while
# the PE still has slack); the rest use ACT Square-accum on the f32 data
GRAM_CHUNKS = {2, 3, 4, 5}
# chunks whose f32->fp8 cast runs on DVE (rest on ACT)
DVE_CAST_CHUNKS = {7, 8, 9, 10}

N_WARMUP_MM = 8  # junk matmuls to lift the PE HAM throttle before real work

AUX_COLS = 512  # fp16 iota row; labels ride a small f32 tensor

_CACHE = {}


def _build():
    """Trace the Bass/Tile program once; returns the compiled Bacc module."""
    if "nc" in _CACHE:
        return _CACHE["nc"]

    import concourse.bacc as bacc
    import concourse.mybir as mybir
    import concourse.tile as tile

    f32 = mybir.dt.float32
    fp8 = mybir.dt.float8e4

    nc = bacc.Bacc("TRN2", debug=False, num_devices=N_CORES)
    x_t = nc.dram_tensor("x", [B_L, D], f32, kind="ExternalInput")
    aux_t = nc.dram_tensor("aux", [128, AUX_COLS], mybir.dt.float16, kind="ExternalInput")
    lab_t = nc.dram_tensor("labx", [128, B_L // 128 + 1], f32, kind="ExternalInput")
    c_t = nc.dram_tensor("centers", [D, D], f32, kind="ExternalInput")
    out_t = nc.dram_tensor("out", [1, 1], f32, kind="ExternalOutput")

    qcs = [r // 128 for r in CHUNK_ROWS]
    toff = [sum(qcs[:i]) for i in range(N_CHUNKS)]  # labf col offset
    goff = [sum(q // 2 for q in qcs[:i]) for i in range(N_CHUNKS)]
    n_groups = B_L // 256  # 32 DoubleRow matmul groups
    gram_groups = sorted(
        goff[ci] + j for ci in GRAM_CHUNKS for j in range(qcs[ci] // 2)
    )
    sumsq_chunks = [ci for ci in range(N_CHUNKS) if ci not in GRAM_CHUNKS]

    with tile.TileContext(nc) as tc:
        with (
            tc.tile_pool(name="misc", bufs=1) as misc_pool,
            tc.tile_pool(name="psum", bufs=1, space="PSUM") as psum_pool,
        ):
            iota_sb16 = misc_pool.tile([128, D], mybir.dt.float16)
            lab_sb = misc_pool.tile([128, B_L // 128 + 1], f32)
            cent_sb = misc_pool.tile([128, NCH, D], f32)
            x32 = [
                None
                if CHUNKS[i][1] == "gpsimd"
                else misc_pool.tile([128, q, D], f32, name=f"x32_{i}")
                for i, q in enumerate(qcs)
            ]
            x8 = [
                misc_pool.tile([128, q, D], fp8, name=f"x8_{i}")
                for i, q in enumerate(qcs)
            ]

            labf_sb = lab_sb[:, 0 : B_L // 128]
            pidx_sb = lab_sb[:, B_L // 128 : B_L // 128 + 1]
            ones_sb = misc_pool.tile([128, 1], f32)

            acc_x2 = misc_pool.tile([128, len(sumsq_chunks)], f32)
            r2acc = misc_pool.tile([128, NCH], f32)
            junk_dve = misc_pool.tile([128, 1], f32)
            junk_act = misc_pool.tile([128, 1], f32)
            r1 = misc_pool.tile([128, 1], f32)
            r1g = misc_pool.tile([128, 1], f32)
            r2 = misc_pool.tile([128, 1], f32)
            total = misc_pool.tile([128, 1], f32)
            res_sb = misc_pool.tile([128, 1], f32)
            eye8 = misc_pool.tile([128, 1, 128], f32)
            warm8 = misc_pool.tile([128, 2, D], fp8)

            S_all = psum_pool.tile([128, NCH, D], f32, name="S_all")
            S_ps = [S_all[:, c, :] for c in range(NCH)]
            # one bank per Gram block: start=True clears has_written at BANK
            # granularity, so accumulation regions must not share a bank
            G_wide = psum_pool.tile([128, NCH, D], f32, name="G_wide")
            G_blk = [G_wide[:, m, 0:128] for m in range(NCH)]
            warm_ps = G_wide[:, 0, :]  # warmup scribbles are cleared by the
            red_ps = G_wide[0:1, 0, 0:2]  # real G start; reduce runs last

            # --- DMA triggers, emission order = semaphore allocation order.
            # sync ring: aux + first sync chunks; scalar ring: its 3 chunks
            # (must never block the ACT engine); then centers on the idle
            # gpsimd queue and the remaining sync chunks (a blocked trigger
            # on the sync/gpsimd engines is harmless).
            x_ap = x_t.ap()
            chunk_lo = []
            lo = 0
            for rows, _ in CHUNKS:
                chunk_lo.append(lo)
                lo += rows

            def x_src(ci):
                lo, rows = chunk_lo[ci], CHUNK_ROWS[ci]
                return x_ap[lo : lo + rows, :].rearrange("(p q) d -> p q d", p=128)

            def x_src_cast(ci):
                # 4-row (8KB) descriptor granularity for the SWDGE cast DMA:
                # big enough that Q7 descriptor-gen keeps up, small enough
                # that packet-granular round-robin shares fairly with HWDGE
                lo, rows = chunk_lo[ci], CHUNK_ROWS[ci]
                return x_ap[lo : lo + rows, :].rearrange(
                    "(p q f) d -> p q (f d)", p=128, f=4
                )

            # emission order = semaphore allocation order: the first ~7
            # in-flight DMAs get sems without blocking; later triggers
            # block only their own (idle) issuing engine until a sem frees
            sync_x = [ci for ci, (_, ring) in enumerate(CHUNKS) if ring == "sync"]
            gps_x = [ci for ci, (_, ring) in enumerate(CHUNKS) if ring == "gpsimd"]
            nc.sync.dma_start(iota_sb16[:], aux_t.ap())
            nc.sync.dma_start(lab_sb[:], lab_t.ap())
            for ci in sync_x[:4]:
                nc.sync.dma_start(x32[ci][:], x_src(ci))
            for ci in gps_x:
                nc.gpsimd.dma_start(
                    x8[ci][:].rearrange("p q d -> p (q d)").rearrange(
                        "p (q v) -> p q v", v=4 * D
                    ),
                    x_src_cast(ci),
                )
            for ci in sync_x[4:]:
                nc.sync.dma_start(x32[ci][:], x_src(ci))
            nc.gpsimd.dma_start(
                cent_sb[:], c_t.ap().rearrange("(n p) d -> p n d", p=128)
            )

            # --- PE warmup on a memset tile (HAM un-throttle) + identity
            nc.vector.memset(warm8[:], 0.0)
            nc.vector.memset(ones_sb[:], 1.0)
            for _ in range(N_WARMUP_MM):
                nc.tensor.matmul(
                    warm_ps[:],
                    lhsT=warm8[:, :, 0:128],
                    rhs=warm8[:],
                    start=True,
                    stop=True,
                    perf_mode=mybir.MatmulPerfMode.DoubleRow,
                )
            # eye8[p, m, q] = (q == p), for extracting Gram diag blocks
            if GRAM_CHUNKS:
                nc.vector.tensor_scalar(
                    out=eye8[:, 0, :],
                    in0=iota_sb16[:, 0:128],
                    scalar1=pidx_sb,
                    scalar2=None,
                    op0=mybir.AluOpType.is_equal,
                )

            # --- main pipeline, chunk order = arrival order
            for ci, (rows, ring) in enumerate(CHUNKS):
                qc = qcs[ci]
                # f32 -> fp8 cast per 512-row slab on the assigned engine
                if ring != "gpsimd":
                    n_sl = (qc + 3) // 4
                    for k in range(n_sl):
                        sl = slice(4 * k, min(4 * k + 4, qc))
                        if ci in DVE_CAST_CHUNKS:
                            nc.vector.tensor_copy(
                                x8[ci][:, sl, :], x32[ci][:, sl, :]
                            )
                        else:
                            nc.scalar.activation(
                                x8[ci][:, sl, :],
                                x32[ci][:, sl, :],
                                mybir.ActivationFunctionType.Copy,
                            )
                if ci not in GRAM_CHUNKS:
                    # sum(x^2) for the chunk on ACT (exact f32)
                    x_sq = x8[ci] if ring == "gpsimd" else x32[ci]
                    x_flat = x_sq[:].rearrange("p q d -> p (q d)")
                    col = sumsq_chunks.index(ci)
                    nc.scalar.activation(
                        junk_act[:].broadcast_to(x_flat.shape),
                        x_flat,
                        mybir.ActivationFunctionType.Square,
                        accum_out=acc_x2[:, col : col + 1],
                    )
                # per 256-row group: one-hot build (DVE) + 4 DoubleRow
                # matmuls into S (+ Gram diag-blocks on the Gram chunks)
                for j in range(qc // 2):
                    g = goff[ci] + j
                    oh = misc_pool.tile([128, 2, D], fp8, tag="oh", bufs=24)
                    for u in range(2):
                        tcol = 2 * g + u
                        nc.vector.tensor_scalar(
                            out=oh[:, u, :],
                            in0=iota_sb16[:],
                            scalar1=labf_sb[:, tcol : tcol + 1],
                            scalar2=None,
                            op0=mybir.AluOpType.is_equal,
                        )
                    for c in range(NCH):
                        nc.tensor.matmul(
                            S_ps[c],
                            lhsT=oh[:, :, c * 128 : (c + 1) * 128],
                            rhs=x8[ci][:, 2 * j : 2 * j + 2, :],
                            start=g == 0,
                            stop=g == n_groups - 1,
                            perf_mode=mybir.MatmulPerfMode.DoubleRow,
                        )
                    if ci in GRAM_CHUNKS:
                        for m in range(NCH):
                            xs = x8[ci][:, 2 * j : 2 * j + 2, m * 128 : (m + 1) * 128]
                            nc.tensor.matmul(
                                G_blk[m],
                                lhsT=xs,
                                rhs=xs,
                                start=g == gram_groups[0],
                                stop=g == gram_groups[-1],
                                perf_mode=mybir.MatmulPerfMode.DoubleRow,
                            )

            # --- tail
            # r1g = sum of Gram diagonals (identity-masked STT over PSUM)
            if GRAM_CHUNKS:
                G_flat = G_wide[:, :, 0:128]
                nc.vector.scalar_tensor_tensor(
                    out=junk_dve[:].broadcast_to(G_flat.shape),
                    in0=G_flat,
                    scalar=1.0,
                    in1=eye8[:].broadcast_to([128, NCH, 128]),
                    op0=mybir.AluOpType.bypass,
                    op1=mybir.AluOpType.mult,
                    accum_out=r1g[:],
                )
            # r2_c = -2*sum_d S[c,d]*C[c,d], one STT per class chunk
            for c in range(NCH):
                nc.vector.scalar_tensor_tensor(
                    out=junk_dve[:].broadcast_to(S_ps[c].shape),
                    in0=S_ps[c],
                    scalar=-2.0,
                    in1=cent_sb[:, c, :],
                    op0=mybir.AluOpType.mult,
                    op1=mybir.AluOpType.mult,
                    accum_out=r2acc[:, c : c + 1],
                )
            nc.vector.tensor_reduce(
                r1[:], acc_x2[:], axis=mybir.AxisListType.X, op=mybir.AluOpType.add
            )
            nc.vector.tensor_reduce(
                r2[:], r2acc[:], axis=mybir.AxisListType.X, op=mybir.AluOpType.add
            )
            nc.vector.tensor_tensor(total[:], r1[:], r2[:], op=mybir.AluOpType.add)
            if GRAM_CHUNKS:
                nc.vector.tensor_tensor(
                    total[:], total[:], r1g[:], op=mybir.AluOpType.add
                )
            # cross-partition reduce on the PE: [1,1] = total^T @ ones
            nc.tensor.matmul(
                red_ps[0:1, 0:1],
                lhsT=total[:],
                rhs=ones_sb,
                start=True,
                stop=True,
                skip_group_check=True,
            )
            nc.vector.tensor_copy(res_sb[0:1, 0:1], red_ps[0:1, 0:1])
            nc.sync.dma_start(out_t.ap(), res_sb[0:1, 0:1])

    nc.compile()
    _CACHE["nc"] = nc
    return nc


def _prep_inputs(x, labels, centers):
    """Shard full inputs into the 8 per-core input maps."""
    x = np.asarray(x, dtype=np.float32)
    labels = np.asarray(labels)
    centers = np.ascontiguousarray(np.asarray(centers, dtype=np.float32))
    in_maps = []
    for cidx in range(N_CORES):
        xs = np.ascontiguousarray(x[cidx * B_L : (cidx + 1) * B_L])
        lab = np.asarray(labels[cidx * B_L : (cidx + 1) * B_L], dtype=np.int64)
        # labf[p, t]: label of the row that lands at (partition p, q-col t),
        # chunk ci contributing qc = rows/128 q-cols, row = lo + p*qc + qq
        cols = []
        lo = 0
        for rows in CHUNK_ROWS:
            qc = rows // 128
            cols.append(lab[lo : lo + rows].reshape(128, qc))
            lo += rows
        labf = np.concatenate(cols, axis=1).astype(np.float32)
        labx = np.zeros((128, B_L // 128 + 1), dtype=np.float32)
        labx[:, 0 : B_L // 128] = labf
        labx[:, B_L // 128] = np.arange(128, dtype=np.float32)  # pidx
        in_maps.append(
            {
                "x": xs,
                "aux": np.tile(np.arange(D, dtype=np.float16), (128, 1)),
                "labx": np.ascontiguousarray(labx),
                "centers": centers,
            }
        )
    return in_maps


def _run(x, labels, centers, trace=False):
    from concourse import bass_utils

    nc = _build()
    in_maps = _prep_inputs(x, labels, centers)
    res = bass_utils.run_bass_kernel_spmd(
        nc, in_maps, core_ids=list(range(N_CORES)), trace=trace
    )
    total = np.float64(0.0)
    for r in res.results:
        total += np.sum(r["out"].astype(np.float64))
    # r3 = sum_c count_c * ||C_c||^2 from the labels histogram (host-side;
    # needs only labels+centers, no x)
    lab = np.asarray(labels).astype(np.int64)
    bc = np.bincount(lab, minlength=D).astype(np.float64)
    c64 = np.asarray(centers, dtype=np.float64)
    total += float(np.dot(bc, np.einsum("cd,cd->c", c64, c64)))
    return np.array(total, dtype=np.float32), res


def kernel(x, labels, centers):
    out, _ = _run(x, labels, centers, trace=False)
    return out


def kernel_traced(x, labels, centers):
    return _run(x, labels, centers, trace=True)
